# revision 2
# baseline (speedup 1.0000x reference)
"""Trainium2 Bass kernel for an 8-head attention layer + FFN (B=2, S=2048,
D=1024, DQK=128, DFF=4096), distributed over 8 NeuronCores.

Sharding: head-parallel attention (1 head per core), ReduceScatter to sum the
per-head attention outputs while scattering tokens, then token-parallel FFN
(512 tokens per core). One collective total.

On-chip layouts keep the contraction dim on partitions throughout:
  xT   [D, S]  per batch         qT,kT [DQK, S]      v [S, D] (t on partitions)
  expT [t, s] tiles              attnT [D, s-chunk]  hT [DFF, tok]
so every matmul is a plain lhsT.T @ rhs with no on-chip transposes. Softmax
runs without max-subtraction (scores are O(1) here); column sums come from an
ones-vector matmul and the 1/sum row is broadcast across partitions with a
K=1 matmul. All matmuls use float32r (full-rate fp32 on TRN2).
"""
import sys

sys.path.insert(0, "/opt/trn_rl_repo")
import numpy as np

B, S, D, H, DQK, DFF = 2, 2048, 1024, 8, 128, 4096
P = 128
SC = 256                 # attention s-chunk width
NSC = S // SC            # 8 s-chunks per batch
TOK = 512                # tokens per core in the FFN phase
NG = (B * S) // TOK      # 8 token groups == 8 cores
NCORES = 8
NT = S // P              # 16 t-blocks
ND = D // P              # 8 d-blocks
NF = DFF // P            # 32 f-blocks
NO = D // P              # 8 output o-blocks for attnT
SCALE = 1.0 / float(np.sqrt(DQK))


def _mask_schedule(mask):
    """Classify each (t-block, s-chunk) tile of the score matrix.

    Returns (sched, mtiles): sched[sc] is a list of (bt, mask_idx) where
    mask_idx is None for fully-unmasked tiles; fully-masked tiles are
    dropped. mtiles[i] is a [P, SC] 0/1 fp32 tile multiplied into exp(s)
    (layout [t, s], matching the on-chip scoresT layout).
    """
    mask = np.asarray(mask, dtype=bool)
    sched = []
    uniq = {}
    mtiles = []
    for sc in range(NSC):
        s0 = sc * SC
        entries = []
        for bt in range(NT):
            sub = mask[s0 : s0 + SC, bt * P : (bt + 1) * P]  # [s, t]
            if sub.all():
                continue
            if not sub.any():
                entries.append((bt, None))
                continue
            tileT = np.where(sub.T, np.float32(0.0), np.float32(1.0)).copy()
            key = tileT.tobytes()
            if key not in uniq:
                uniq[key] = len(mtiles)
                mtiles.append(tileT)
            entries.append((bt, uniq[key]))
        sched.append(entries)
    return sched, mtiles


def _build(sched, n_mask):
    import concourse.mybir as mybir
    import concourse.tile as tile
    from concourse import bacc

    F32 = mybir.dt.float32
    F32R = mybir.dt.float32r
    AF = mybir.ActivationFunctionType
    OP = mybir.AluOpType

    nc = bacc.Bacc("TRN2", target_bir_lowering=False, debug=False,
                   num_devices=NCORES)

    xT_in = nc.dram_tensor("xT", [B, D, S], F32R, kind="ExternalInput")
    wqT_in = nc.dram_tensor("wqT", [D, DQK], F32R, kind="ExternalInput")
    wkT_in = nc.dram_tensor("wkT", [D, DQK], F32R, kind="ExternalInput")
    wvT_in = nc.dram_tensor("wvT", [D, D], F32R, kind="ExternalInput")
    w1T_in = nc.dram_tensor("w1T", [D, DFF], F32R, kind="ExternalInput")
    w2T_in = nc.dram_tensor("w2T", [DFF, D], F32R, kind="ExternalInput")
    b1_in = nc.dram_tensor("b1c", [P, NF], F32, kind="ExternalInput")
    b2_in = nc.dram_tensor("b2c", [P, ND], F32, kind="ExternalInput")
    mt_in = nc.dram_tensor("mtiles", [max(n_mask, 1), P, SC], F32,
                           kind="ExternalInput")
    onec_in = nc.dram_tensor("onec", [P, 1], F32R, kind="ExternalInput")
    oner_in = nc.dram_tensor("oner", [1, P], F32, kind="ExternalInput")
    xTg_in = nc.dram_tensor("xTg", [D, TOK], F32, kind="ExternalInput")
    outT = nc.dram_tensor("outT", [D, TOK], F32, kind="ExternalOutput")

    # [ND, P, *] views with the d-block index explicit
    xT_r = xT_in.rearrange("b (o p) s -> b o p s", p=P)
    wqT_r = wqT_in.rearrange("(o p) e -> o p e", p=P)
    wkT_r = wkT_in.rearrange("(o p) e -> o p e", p=P)
    wvT_r = wvT_in.rearrange("(o p) d -> o p d", p=P)
    w1T_r = w1T_in.rearrange("(o p) f -> o p f", p=P)
    w2T_r = w2T_in.rearrange("(o p) d -> o p d", p=P)
    xTg_r = xTg_in.rearrange("(o p) t -> o p t", p=P)
    outT_r = outT.rearrange("(o p) t -> o p t", p=P)

    with tile.TileContext(nc) as tc:
        with (
            tc.tile_pool(name="consts", bufs=1) as consts,
            tc.tile_pool(name="dram", bufs=1, space="DRAM") as dram,
        ):
            ones_col = consts.tile([P, 1], F32R, tag="onec")
            nc.sync.dma_start(ones_col[:], onec_in[:])
            ones_row = consts.tile([1, P], F32, tag="oner")
            nc.sync.dma_start(ones_row[:], oner_in[:])
            b1_sb = consts.tile([P, NF], F32, tag="b1")
            nc.sync.dma_start(b1_sb[:], b1_in[:])
            b2_sb = consts.tile([P, ND], F32, tag="b2")
            nc.sync.dma_start(b2_sb[:], b2_in[:])
            mt_sb = []
            for i in range(n_mask):
                t = consts.tile([P, SC], F32, tag=f"mt{i}")
                nc.sync.dma_start(t[:], mt_in[i])
                mt_sb.append(t)

            cc_in = dram.tile([NG, D, TOK], F32, tag="cc_in")
            cc_out = dram.tile([D, TOK], F32, tag="cc_out")

            # ---------------- attention (head-parallel) ----------------
            with (
                tc.tile_pool(name="wqk", bufs=1) as wqk,
                tc.tile_pool(name="wv", bufs=1) as wvp,
                tc.tile_pool(name="xt", bufs=2) as xtp,
                tc.tile_pool(name="qk", bufs=1) as qkp,
                tc.tile_pool(name="vp", bufs=1) as vp,
                tc.tile_pool(name="ep", bufs=18) as ep,
                tc.tile_pool(name="rbp", bufs=2) as rbp,
                tc.tile_pool(name="aop", bufs=4) as aop,
                tc.tile_pool(name="ps_pr", bufs=2, space="PSUM") as ps_pr,
                tc.tile_pool(name="ps_sc", bufs=2, space="PSUM") as ps_sc,
                tc.tile_pool(name="ps_sum", bufs=1, space="PSUM") as ps_sum,
                tc.tile_pool(name="ps_rb", bufs=1, space="PSUM") as ps_rb,
                tc.tile_pool(name="ps_at", bufs=2, space="PSUM") as ps_at,
            ):
                wq_t = []
                wk_t = []
                wv_t = []
                for do in range(ND):
                    wq = wqk.tile([P, DQK], F32R, tag=f"wq{do}")
                    nc.sync.dma_start(wq[:], wqT_r[do])
                    wq_t.append(wq)
                    wk = wqk.tile([P, DQK], F32R, tag=f"wk{do}")
                    nc.sync.dma_start(wk[:], wkT_r[do])
                    wk_t.append(wk)
                    wv = wvp.tile([P, D], F32R, tag=f"wv{do}")
                    nc.sync.dma_start(wv[:], wvT_r[do])
                    wv_t.append(wv)

                for b in range(B):
                    # --- projections: qT/kT [DQK, S], v tiles [P(t), D] ---
                    qT = qkp.tile([P, S], F32R, tag="qT")
                    kT = qkp.tile([P, S], F32R, tag="kT")
                    v_t = [vp.tile([P, D], F32R, tag=f"v{to}", name=f"v{to}") for to in range(NT)]
                    for tch in range(4):  # 512-token chunks of S
                        sl = slice(tch * 512, (tch + 1) * 512)
                        xts = []
                        for do in range(ND):
                            xt = xtp.tile([P, 512], F32R, tag=f"xt{do}")
                            nc.sync.dma_start(xt[:], xT_r[b, do, :, sl])
                            xts.append(xt)
                        qps = ps_pr.tile([P, 512], F32, tag="pr")
                        for do in range(ND):
                            nc.tensor.matmul(qps[:], wq_t[do][:], xts[do][:],
                                             start=(do == 0), stop=(do == ND - 1))
                        nc.vector.tensor_copy(qT[:, sl], qps[:])
                        kps = ps_pr.tile([P, 512], F32, tag="pr")
                        for do in range(ND):
                            nc.tensor.matmul(kps[:], wk_t[do][:], xts[do][:],
                                             start=(do == 0), stop=(do == ND - 1))
                        nc.vector.tensor_copy(kT[:, sl], kps[:])
                        for ti in range(4):  # t-blocks within this chunk
                            to = tch * 4 + ti
                            tsl = slice(ti * P, (ti + 1) * P)
                            for oc in range(2):
                                osl = slice(oc * 512, (oc + 1) * 512)
                                vps = ps_pr.tile([P, 512], F32, tag="pr")
                                for do in range(ND):
                                    nc.tensor.matmul(
                                        vps[:], xts[do][:, tsl], wv_t[do][:, osl],
                                        start=(do == 0), stop=(do == ND - 1))
                                nc.vector.tensor_copy(v_t[to][:, osl], vps[:])

                    # --- scores / softmax / attnT per s-chunk ---
                    for sc in range(NSC):
                        ssl = slice(sc * SC, (sc + 1) * SC)
                        entries = sched[sc]
                        e_sb = {}
                        sums = ps_sum.tile([1, SC], F32, tag="sum")
                        for i, (bt, mi) in enumerate(entries):
                            sp = ps_sc.tile([P, SC], F32, tag="sc")
                            nc.tensor.matmul(
                                sp[:], kT[:, bt * P:(bt + 1) * P], qT[:, ssl],
                                start=True, stop=True)
                            e = ep.tile([P, SC], F32R, tag="e")
                            nc.scalar.activation(e[:], sp[:], AF.Exp, scale=SCALE)
                            if mi is not None:
                                nc.vector.tensor_tensor(e[:], e[:], mt_sb[mi][:],
                                                        OP.mult)
                            e_sb[bt] = e
                            nc.tensor.matmul(sums[:], ones_col[:], e[:],
                                             start=(i == 0),
                                             stop=(i == len(entries) - 1))
                        rec = rbp.tile([1, SC], F32, tag="rec")
                        nc.vector.reciprocal(rec[:], sums[:])
                        rbps = ps_rb.tile([P, SC], F32, tag="rb")
                        nc.tensor.matmul(rbps[:], ones_row[:], rec[:],
                                         start=True, stop=True)
                        rb_sb = rbp.tile([P, SC], F32, tag="rb_sb")
                        nc.vector.tensor_copy(rb_sb[:], rbps[:])

                        g = (b * S + sc * SC) // TOK
                        off = (sc * SC) % TOK
                        for oc in range(NO):
                            ap = ps_at.tile([P, SC], F32, tag="at")
                            for i, (bt, _mi) in enumerate(entries):
                                nc.tensor.matmul(
                                    ap[:], v_t[bt][:, oc * P:(oc + 1) * P],
                                    e_sb[bt][:],
                                    start=(i == 0), stop=(i == len(entries) - 1))
                            ao = aop.tile([P, SC], F32, tag="ao")
                            nc.vector.tensor_tensor(ao[:], ap[:], rb_sb[:], OP.mult)
                            nc.sync.dma_start(
                                cc_in[g, oc * P:(oc + 1) * P, off:off + SC], ao[:])

            # ---------------- ReduceScatter over heads/tokens ----------------
            nc.gpsimd.collective_compute(
                "ReduceScatter",
                mybir.AluOpType.add,
                replica_groups=[list(range(NCORES))],
                ins=[cc_in.opt()],
                outs=[cc_out.opt()],
            )

            # ---------------- FFN (token-parallel) ----------------
            cc_out_r = cc_out.rearrange("(o p) t -> o p t", p=P)
            with (
                tc.tile_pool(name="ldp", bufs=2) as ldp,
                tc.tile_pool(name="resp", bufs=1) as resp,
                tc.tile_pool(name="hp", bufs=1) as hp,
                tc.tile_pool(name="w1p", bufs=2) as w1p,
                tc.tile_pool(name="w2p", bufs=4) as w2p,
                tc.tile_pool(name="outp", bufs=4) as outp,
                tc.tile_pool(name="ps_f1", bufs=2, space="PSUM") as ps_f1,
                tc.tile_pool(name="ps_f2", bufs=2, space="PSUM") as ps_f2,
            ):
                res1 = []
                for do in range(ND):
                    xg = ldp.tile([P, TOK], F32, tag="xg")
                    nc.sync.dma_start(xg[:], xTg_r[do])
                    co = ldp.tile([P, TOK], F32, tag="co")
                    nc.sync.dma_start(co[:], cc_out_r[do])
                    r1 = resp.tile([P, TOK], F32R, tag=f"r1_{do}")
                    nc.vector.tensor_add(r1[:], xg[:], co[:])
                    res1.append(r1)

                h_t = []
                for fo in range(NF):
                    w1ts = []
                    for do in range(ND):
                        w1t = w1p.tile([P, P], F32R, tag=f"w1_{do}")
                        nc.sync.dma_start(w1t[:], w1T_r[do, :, fo * P:(fo + 1) * P])
                        w1ts.append(w1t)
                    hps = ps_f1.tile([P, TOK], F32, tag="f1")
                    for do in range(ND):
                        nc.tensor.matmul(hps[:], w1ts[do][:], res1[do][:],
                                         start=(do == 0), stop=(do == ND - 1))
                    ht = hp.tile([P, TOK], F32R, tag=f"h_{fo}")
                    nc.scalar.activation(ht[:], hps[:], AF.Relu,
                                         bias=b1_sb[:, fo:fo + 1])
                    h_t.append(ht)

                for do in range(ND):
                    ops = ps_f2.tile([P, TOK], F32, tag="f2")
                    for fo in range(NF):
                        w2t = w2p.tile([P, P], F32R, tag="w2")
                        nc.sync.dma_start(w2t[:], w2T_r[fo, :, do * P:(do + 1) * P])
                        nc.tensor.matmul(ops[:], w2t[:], h_t[fo][:],
                                         start=(fo == 0), stop=(fo == NF - 1))
                    o1 = outp.tile([P, TOK], F32, tag="o1")
                    nc.vector.tensor_add(o1[:], ops[:], res1[do][:])
                    o2 = outp.tile([P, TOK], F32, tag="o2")
                    nc.vector.tensor_scalar_add(o2[:], o1[:], b2_sb[:, do:do + 1])
                    nc.sync.dma_start(outT_r[do], o2[:])

    nc.compile()
    return nc


_CACHE = {}


def kernel(encodings, Wq, Wk, Wv, W1, b1, W2, b2, mask):
    from concourse.bass_utils import run_bass_kernel_spmd

    x = np.ascontiguousarray(np.asarray(encodings, dtype=np.float32))
    sched, mtiles = _mask_schedule(mask)
    n_mask = len(mtiles)

    key = (tuple(tuple(e) for e in sched), n_mask)
    if key not in _CACHE:
        _CACHE[key] = _build(sched, n_mask)
    nc = _CACHE[key]

    xT = np.ascontiguousarray(x.transpose(0, 2, 1))                  # [B, D, S]
    w1T = np.ascontiguousarray(np.asarray(W1, np.float32).T)         # [D, DFF]
    w2T = np.ascontiguousarray(np.asarray(W2, np.float32).T)         # [DFF, D]
    b1c = np.ascontiguousarray(np.asarray(b1, np.float32).reshape(NF, P).T)
    b2c = np.ascontiguousarray(np.asarray(b2, np.float32).reshape(ND, P).T)
    mt = (np.stack(mtiles) if n_mask else np.zeros((1, P, SC), np.float32))
    onec = np.ones((P, 1), np.float32)
    oner = np.ones((1, P), np.float32)

    xflat = x.reshape(B * S, D)
    in_maps = []
    for c in range(NCORES):
        in_maps.append({
            "xT": xT,
            "wqT": np.ascontiguousarray(np.asarray(Wq[c], np.float32).T),
            "wkT": np.ascontiguousarray(np.asarray(Wk[c], np.float32).T),
            "wvT": np.ascontiguousarray(np.asarray(Wv[c], np.float32).T),
            "w1T": w1T,
            "w2T": w2T,
            "b1c": b1c,
            "b2c": b2c,
            "mtiles": mt,
            "onec": onec,
            "oner": oner,
            "xTg": np.ascontiguousarray(xflat[c * TOK:(c + 1) * TOK].T),
        })

    res = run_bass_kernel_spmd(nc, in_maps, core_ids=list(range(NCORES)))
    out = np.empty((B * S, D), np.float32)
    for c in range(NCORES):
        out[c * TOK:(c + 1) * TOK] = res.results[c]["outT"].T
    kernel.last_results = res
    return out.reshape(B, S, D)
